# revision 2
# baseline (speedup 1.0000x reference)
"""Trainium2 Bass kernel for nn_CMAF (cross-modal attention fusion block), v2.

Feature-major layout: [128 features x 1024 samples] tiles, weight-stationary
bf16 matmuls. Key deltas vs v1:
 - LayerNorm rsqrt via single-op AF.Rsqrt (reciprocal_sqrt table set); the
   two ACT table sets per tick are {gelu,tanh,identity,square} and
   {reciprocal_sqrt,square,identity} -> still 2 table loads per tick.
 - attention 2-way softmax: a0 = 0.5 + 0.5*tanh(d*ISQ/2) (tanh lives in the
   gelu set), d accumulated by 12 packed 32x32 tile_position matmuls
   (per-head reduce+broadcast), with v/out-proj weights partition-rolled by
   32n per branch so each branch's head sums land in distinct array columns.
 - out-projection runs once on (a0*dv + v1).
 - FFN first matmul consumes x1 directly (no rb2 rescale of W1 outputs).
 - gate softmax over differences d_n = g_n - g_2: e^d via tanh identity
   e^d = (1+t)/(1-t), fused = x2_2 + a0*(x2_0-x2_2) + a1*(x2_1-x2_2).
 - output written feature-major bf16 [128, Bc] to DRAM; transposed and
   upcast on host.

Data parallel over 8 NeuronCores: 8192 samples each.
"""

import numpy as np
import ml_dtypes

import concourse.bass as bass
import concourse.mybir as mybir
from concourse.tile import TileContext
from concourse.vector_clock import ScopedClock
from concourse.bass_utils import run_bass_kernel_spmd

F32 = mybir.dt.float32
BF16 = mybir.dt.bfloat16
AL = mybir.AluOpType
AF = mybir.ActivationFunctionType
NPBF = ml_dtypes.bfloat16

D = 128
SP = 1280
FFN = 256
NB = 3
DH = 32
KV_IDX = ((1, 2), (0, 2), (0, 1))
NCORES = 8
BLK = 1024
MMN = 512
EPS = 1e-5
ISQ = float(1.0 / np.sqrt(DH))


def _patch_tile_drain():
    """walrus rejects >4 sem waits on one instruction; Tile's tail drain
    carries one wait per logical proc. Re-emit as standalone wait_ge."""
    TC = TileContext
    if getattr(TC, "_drain_patched", False):
        return

    def patched(self, tick_clock, wait_clock):
        nop_inst = self.nc.sync.nop()
        wait_clock.add_sem_waits(
            nop_inst.ins, ScopedClock({None: tick_clock.global_clock})
        )
        d = nop_inst.ins
        si = d.sync_info
        waits = list(si.on_wait) if si is not None else []
        if len(waits) > 4:
            si.on_wait = []
            d.sync_info = si
            name2sem = {s.name: s for s in self.sems.allocated().values()}
            for w in waits:
                sem = name2sem.get(w.ant_name)
                if sem is None:
                    raise RuntimeError(f"drain patch: unknown sem {w.ant_name}")
                self.nc.sync.wait_ge(sem, w.wait_value)
        self.nc.sync.drain()
        self.nc.all_engine_barrier()
        popped = self.nc._tile_sem_poison_stack.pop()
        assert popped is self._sem_poison
        self.nc.clear_and_free_semaphores(list(self.sems.allocated().values()))
        self.nc.all_engine_barrier()

    TC._drain_and_barrier = patched
    TC._drain_patched = True


def _fix_wait_overflow(nc):
    """Move excess sync-waits onto same-engine NOPs before the instruction."""
    LIMITS = {}
    DEFAULT_LIM = 1
    for fn in nc.m.functions:
        for bb in fn.blocks:
            insts = list(bb.instructions)
            out = []
            changed = False
            for inst in insts:
                si = getattr(inst, "sync_info", None)
                w = list(si.on_wait) if si is not None and si.on_wait else []
                lim = LIMITS.get(type(inst).__name__, DEFAULT_LIM)
                if len(w) > lim:
                    excess = w[lim:]
                    keep = w[:lim]
                    eng = nc.engines[inst.engine]
                    nops = []
                    for i in range(0, len(excess), 1):
                        chunk = excess[i:i + 1]
                        nop_bi = eng.nop()
                        nop_inst = nop_bi.ins
                        cb = nc.cur_bb.bb
                        cb.instructions = [x for x in cb.instructions
                                           if x.name != nop_inst.name]
                        import bass_rust
                        nop_inst.sync_info = bass_rust.SyncInfo(
                            on_wait=chunk, on_update=[])
                        nops.append(nop_inst)
                    si.on_wait = keep
                    inst.sync_info = si
                    out.extend(nops)
                    changed = True
                out.append(inst)
            if changed:
                bb.instructions = out


def prep_weights(inp):
    f64 = np.float64
    C = np.eye(D, dtype=f64) - 1.0 / D

    def bf(a):
        return np.ascontiguousarray(np.asarray(a, dtype=np.float32)).astype(NPBF)

    def f32(a):
        return np.ascontiguousarray(np.asarray(a), dtype=np.float32)

    w = {}
    wsp = C @ np.asarray(inp["proj_w_spatial"], f64)          # [128,1280]
    w["wspT"] = bf(np.concatenate(
        [wsp[:, c * D:(c + 1) * D].T for c in range(10)], axis=1))
    wgf = np.stack([C @ np.asarray(inp["proj_w_gf"], f64)[i] for i in range(2)])
    w["wgfT"] = bf(np.concatenate([wgf[i].T for i in range(2)], axis=1))
    w["bc"] = f32((C @ np.asarray(inp["proj_b"], f64).T))      # [128,3]

    emb = np.asarray(inp["mod_emb"], f64)                      # [3,128]
    emb_c = emb - emb.mean(axis=1, keepdims=True)

    ipw = np.asarray(inp["in_proj_w"], f64)                    # [3, 384, 128]
    Wq = ipw[:, :D]
    Wk = ipw[:, D:2 * D]
    Wv = ipw[:, 2 * D:]
    ow = np.asarray(inp["out_proj_w"], f64)                    # [3,128,128]
    ob = np.asarray(inp["out_proj_b"], f64)                    # [3,128]

    w["wqT"] = bf(np.concatenate([Wq[n].T for n in range(NB)], axis=1))
    w["wkT"] = bf(np.concatenate([Wk[n].T for n in range(NB)], axis=1))
    w["wvT"] = bf(np.concatenate(
        [np.roll(Wv[n], 32 * n, axis=0).T for n in range(NB)], axis=1))
    owc = [C @ ow[n] for n in range(NB)]
    w["owT"] = bf(np.concatenate(
        [np.roll(owc[n], 32 * n, axis=1).T for n in range(NB)], axis=1))

    w["qb"] = f32(np.stack([Wq[n] @ emb[n] for n in range(NB)], axis=1))
    w["demb"] = f32(np.stack(
        [emb[KV_IDX[n][0]] - emb[KV_IDX[n][1]] for n in range(NB)], axis=1))
    ube = [C @ ob[n] + C @ (ow[n] @ (Wv[n] @ emb[KV_IDX[n][1]])) + emb_c[n]
           for n in range(NB)]
    w["ube"] = f32(np.stack(ube, axis=1))

    w1 = np.asarray(inp["ffn_w1"], f64)                        # [3,256,128]
    w["w1T"] = bf(np.concatenate(
        [w1[n][c * D:(c + 1) * D].T for n in range(NB) for c in range(2)],
        axis=1))
    w["b1"] = f32(np.asarray(inp["ffn_b1"], f64).reshape(NB * 2, D).T)
    w2 = [C @ np.asarray(inp["ffn_w2"], f64)[n] for n in range(NB)]  # [128,256]
    w["w2T"] = bf(np.concatenate(
        [w2[n][:, c * D:(c + 1) * D].T for n in range(NB) for c in range(2)],
        axis=1))
    w["b2c"] = f32(np.stack([C @ np.asarray(inp["ffn_b2"], f64)[n]
                             for n in range(NB)], axis=1))

    gw = np.asarray(inp["gate_w"], f64).reshape(NB, NB, D)     # [out n, m, d]
    gb = np.asarray(inp["gate_b"], f64)
    gwd = np.stack([gw[n] - gw[2] for n in range(2)])          # [2, 3, 128]
    w["gwdT"] = bf(np.concatenate(
        [gwd[:, m, :].T for m in range(NB)], axis=1))          # [128, 6]
    w["gbdh"] = f32(0.5 * (gb[:2] - gb[2]).reshape(2, 1))      # tanh bias

    w["onesT"] = bf(np.full((D, D), 1.0 / D))
    w["hones"] = bf(np.ones((D, 32)))
    w["ones3z"] = bf(np.ones((NB, 32)))
    esel = np.zeros((2, 2 * D), dtype=np.float32)
    esel[0, :D] = 1.0
    esel[1, D:] = 1.0
    w["esel"] = bf(esel)
    w["epsv"] = np.full((D, 1), EPS, dtype=np.float32)
    w["zerov"] = np.zeros((D, 1), dtype=np.float32)

    assert np.allclose(inp["proj_ln_g"], 1) and np.allclose(inp["proj_ln_b"], 0)
    assert np.allclose(inp["attn_ln_g"], 1) and np.allclose(inp["attn_ln_b"], 0)
    assert np.allclose(inp["ffn_ln_g"], 1) and np.allclose(inp["ffn_ln_b"], 0)
    assert np.allclose(inp["in_proj_b"], 0)
    return w


WEIGHT_SPECS = {
    "wspT": ((D, 10 * D), BF16), "wgfT": ((D, 2 * D), BF16),
    "bc": ((D, NB), F32),
    "wqT": ((D, NB * D), BF16), "wkT": ((D, NB * D), BF16),
    "wvT": ((D, NB * D), BF16), "owT": ((D, NB * D), BF16),
    "qb": ((D, NB), F32), "demb": ((D, NB), F32), "ube": ((D, NB), F32),
    "w1T": ((D, NB * 2 * D), BF16), "b1": ((D, NB * 2), F32),
    "w2T": ((D, NB * 2 * D), BF16), "b2c": ((D, NB), F32),
    "gwdT": ((D, 2 * NB), BF16), "gbdh": ((2, 1), F32),
    "onesT": ((D, D), BF16), "hones": ((D, 32), BF16),
    "ones3z": ((NB, 32), BF16), "esel": ((2, 2 * D), BF16),
    "epsv": ((D, 1), F32), "zerov": ((D, 1), F32),
}


def build_program(Bc, repeat=1):
    nc = bass.Bass()
    xsp = nc.dram_tensor("x_spatial", [Bc, SP], BF16, kind="ExternalInput")
    xg = nc.dram_tensor("x_gradient", [Bc, D], BF16, kind="ExternalInput")
    xf = nc.dram_tensor("x_frequency", [Bc, D], BF16, kind="ExternalInput")
    wd = {k: nc.dram_tensor(k, list(s[0]), s[1], kind="ExternalInput")
          for k, s in WEIGHT_SPECS.items()}
    # feature-major output; host transposes
    out = nc.dram_tensor("outT", [D, Bc], BF16, kind="ExternalOutput")

    nblk = Bc // BLK
    assert Bc % BLK == 0

    with TileContext(nc) as tc, nc.allow_low_precision(reason="bf16 kernel"):
        with (
            tc.tile_pool(name="wp", bufs=1) as wp,
            tc.tile_pool(name="xin", bufs=2) as xin,
            tc.tile_pool(name="pp", bufs=2) as pp,
            tc.tile_pool(name="sc", bufs=1) as sc,
            tc.tile_pool(name="sc2", bufs=2) as sc2,
            tc.tile_pool(name="ps", bufs=4, space="PSUM") as psp,
        ):
            W = {}
            for k, s in WEIGHT_SPECS.items():
                W[k] = wp.tile(list(s[0]), s[1], tag=k, name=k)
                nc.gpsimd.dma_start(W[k][:], wd[k][:])
            # persistent gate-exp tile: row 2 stays 1.0
            e2ext = wp.tile([NB, BLK], BF16, tag="e2ext", name="e2ext")
            nc.vector.memset(e2ext[:], 1.0)

            def mm(out_ap, lhsT, rhs, start=True, stop=True):
                for h in range(BLK // MMN):
                    nc.tensor.matmul(out_ap[:, h * MMN:(h + 1) * MMN], lhsT,
                                     rhs[:, h * MMN:(h + 1) * MMN],
                                     start=start, stop=stop)

            def phase0(b):
                r0 = (b % nblk) * BLK
                st = {}
                xspT = xin.tile([D, 10 * BLK], BF16, tag="xspT")
                nc.sync.dma_start(
                    xspT[:].rearrange("p (c n) -> p c n", c=10),
                    xsp[r0:r0 + BLK, :], transpose=True)
                st["xspT"] = xspT
                st["xgT"] = xin.tile([D, BLK], BF16, tag="xgT")
                nc.sync.dma_start(st["xgT"][:], xg[r0:r0 + BLK, :], transpose=True)
                st["xfT"] = xin.tile([D, BLK], BF16, tag="xfT")
                nc.sync.dma_start(st["xfT"][:], xf[r0:r0 + BLK, :], transpose=True)
                return st

            def phase1(st):
                # projections + LN1 -> Pall (no emb), dPall (emb diff folded)
                z_ps = []
                zs = psp.tile([D, BLK], F32, tag="ps")
                for c in range(10):
                    mm(zs[:], W["wspT"][:, c * D:(c + 1) * D],
                       st["xspT"][:, c * BLK:(c + 1) * BLK],
                       start=(c == 0), stop=(c == 9))
                z_ps.append(zs)
                for i, key in ((0, "xgT"), (1, "xfT")):
                    zt = psp.tile([D, BLK], F32, tag="ps")
                    mm(zt[:], W["wgfT"][:, i * D:(i + 1) * D], st[key][:])
                    z_ps.append(zt)
                Pall = pp.tile([D, NB * BLK], BF16, tag="Pall")
                sqa = sc.tile([D, BLK], BF16, tag="sq1")
                for n in range(NB):
                    nc.scalar.activation(sqa[:], z_ps[n][:], AF.Square,
                                         bias=W["bc"][:, n:n + 1])
                    mq = psp.tile([D, BLK], F32, tag="ps")
                    mm(mq[:], W["onesT"][:], sqa[:])
                    rb = sc.tile([D, BLK], BF16, tag=f"rb1_{n}")
                    nc.scalar.activation(rb[:], mq[:], AF.Rsqrt,
                                         bias=W["epsv"][:, 0:1])
                    nc.vector.scalar_tensor_tensor(
                        Pall[:, n * BLK:(n + 1) * BLK], z_ps[n][:],
                        W["bc"][:, n:n + 1], rb[:], AL.add, AL.mult)
                dPall = pp.tile([D, NB * BLK], BF16, tag="dPall")
                for n in range(NB):
                    s0, s1 = KV_IDX[n]
                    nc.vector.scalar_tensor_tensor(
                        dPall[:, n * BLK:(n + 1) * BLK],
                        Pall[:, s0 * BLK:(s0 + 1) * BLK],
                        W["demb"][:, n:n + 1],
                        Pall[:, s1 * BLK:(s1 + 1) * BLK],
                        AL.add, AL.subtract)
                st["Pall"] = Pall
                st["dPall"] = dPall

            def phase2a(st):
                Pall, dPall = st["Pall"], st["dPall"]
                uall = sc.tile([D, NB * BLK], BF16, tag="uall")
                for n in range(NB):
                    Pn = Pall[:, n * BLK:(n + 1) * BLK]
                    dPn = dPall[:, n * BLK:(n + 1) * BLK]
                    s1 = KV_IDX[n][1]
                    q_ps = psp.tile([D, BLK], F32, tag="ps")
                    mm(q_ps[:], W["wqT"][:, n * D:(n + 1) * D], Pn)
                    dk_ps = psp.tile([D, BLK], F32, tag="ps")
                    mm(dk_ps[:], W["wkT"][:, n * D:(n + 1) * D], dPn)
                    qx = sc.tile([D, BLK], BF16, tag="qx")
                    nc.scalar.activation(qx[:], q_ps[:], AF.Identity,
                                         bias=W["qb"][:, n:n + 1])
                    t0 = sc.tile([D, BLK], BF16, tag="t0")
                    nc.vector.tensor_tensor(t0[:], qx[:], dk_ps[:], AL.mult)
                    d_ps = psp.tile([D, BLK], F32, tag="ps")
                    for h in range(4):
                        c = (h + n) % 4
                        for k in range(BLK // MMN):
                            nc.tensor.matmul(
                                d_ps[32 * c:32 * c + 32, k * MMN:(k + 1) * MMN],
                                W["hones"][32 * h:32 * h + 32, :],
                                t0[32 * h:32 * h + 32, k * MMN:(k + 1) * MMN],
                                start=True, stop=True,
                                tile_position=(32 * h, 32 * c))
                    th = sc.tile([D, BLK], BF16, tag="th")
                    nc.scalar.activation(th[:], d_ps[:], AF.Tanh,
                                         bias=W["zerov"][:, 0:1],
                                         scale=ISQ * 0.5)
                    a0 = sc.tile([D, BLK], BF16, tag="a0")
                    nc.vector.tensor_scalar(a0[:], th[:], 0.5, 0.5,
                                            AL.mult, AL.add)
                    dv_ps = psp.tile([D, BLK], F32, tag="ps")
                    mm(dv_ps[:], W["wvT"][:, n * D:(n + 1) * D], dPn)
                    v1_ps = psp.tile([D, BLK], F32, tag="ps")
                    mm(v1_ps[:], W["wvT"][:, n * D:(n + 1) * D],
                       Pall[:, s1 * BLK:(s1 + 1) * BLK])
                    tp = sc.tile([D, BLK], BF16, tag="tp")
                    nc.vector.tensor_tensor(tp[:], a0[:], dv_ps[:], AL.mult)
                    tpv = sc.tile([D, BLK], BF16, tag="tpv")
                    nc.vector.tensor_tensor(tpv[:], tp[:], v1_ps[:], AL.add)
                    o_ps = psp.tile([D, BLK], F32, tag="ps")
                    mm(o_ps[:], W["owT"][:, n * D:(n + 1) * D], tpv[:])
                    nc.vector.scalar_tensor_tensor(
                        uall[:, n * BLK:(n + 1) * BLK], o_ps[:],
                        W["ube"][:, n:n + 1], Pn, AL.add, AL.add)
                st["uall"] = uall

            def phase2b(st):
                uall = st["uall"]
                squ = sc.tile([D, NB * BLK], BF16, tag="squ")
                nc.scalar.activation(squ[:], uall[:], AF.Square,
                                     bias=W["zerov"][:, 0:1])
                x1all = sc2.tile([D, NB * BLK], BF16, tag="x1all")
                rb2 = sc.tile([D, NB * BLK], BF16, tag="rb2")
                for n in range(NB):
                    mq = psp.tile([D, BLK], F32, tag="ps")
                    mm(mq[:], W["onesT"][:], squ[:, n * BLK:(n + 1) * BLK])
                    sd2 = sc.tile([D, BLK], BF16, tag="sd2", name="sd2")
                    nc.scalar.activation(sd2[:], mq[:],
                                         AF.Sqrt, bias=W["epsv"][:, 0:1])
                    nc.vector.reciprocal(rb2[:, n * BLK:(n + 1) * BLK], sd2[:])
                    nc.vector.tensor_tensor(
                        x1all[:, n * BLK:(n + 1) * BLK],
                        uall[:, n * BLK:(n + 1) * BLK],
                        rb2[:, n * BLK:(n + 1) * BLK], AL.mult)
                st["x1all"] = x1all

            def phase3a(st):
                x1all = st["x1all"]
                gsb = sc.tile([D, NB * 2 * BLK], BF16, tag="gsb")
                for n in range(NB):
                    x1n = x1all[:, n * BLK:(n + 1) * BLK]
                    for c in range(2):
                        j = 2 * n + c
                        h_ps = psp.tile([D, BLK], F32, tag="ps")
                        mm(h_ps[:], W["w1T"][:, j * D:(j + 1) * D], x1n)
                        nc.scalar.activation(gsb[:, j * BLK:(j + 1) * BLK],
                                             h_ps[:], AF.Gelu,
                                             bias=W["b1"][:, j:j + 1])
                st["gsb"] = gsb

            def phase3b(st):
                x1all, gsb = st["x1all"], st["gsb"]
                x2all = sc2.tile([D, NB * BLK], BF16, tag="x2all")
                sq3 = sc.tile([D, BLK], BF16, tag="sq3")
                for n in range(NB):
                    f_ps = psp.tile([D, BLK], F32, tag="ps")
                    for c in range(2):
                        j = 2 * n + c
                        mm(f_ps[:], W["w2T"][:, j * D:(j + 1) * D],
                           gsb[:, j * BLK:(j + 1) * BLK],
                           start=(c == 0), stop=(c == 1))
                    x2p = sc.tile([D, BLK], BF16, tag=f"x2p{n}")
                    nc.vector.tensor_tensor(
                        x2p[:], x1all[:, n * BLK:(n + 1) * BLK], f_ps[:],
                        AL.add)
                    nc.scalar.activation(sq3[:], x2p[:], AF.Square,
                                         bias=W["b2c"][:, n:n + 1])
                    mq = psp.tile([D, BLK], F32, tag="ps")
                    mm(mq[:], W["onesT"][:], sq3[:])
                    rb3 = sc.tile([D, BLK], BF16, tag="rb3")
                    nc.scalar.activation(rb3[:], mq[:], AF.Rsqrt,
                                         bias=W["epsv"][:, 0:1])
                    nc.vector.scalar_tensor_tensor(
                        x2all[:, n * BLK:(n + 1) * BLK], x2p[:],
                        W["b2c"][:, n:n + 1], rb3[:], AL.add, AL.mult)
                st["x2all"] = x2all

            def phase4a(st):
                x2all = st["x2all"]
                g_ps = psp.tile([D, BLK], F32, tag="ps")
                for m in range(NB):
                    mm(g_ps[0:2, :], W["gwdT"][:, m * 2:(m + 1) * 2],
                       x2all[:, m * BLK:(m + 1) * BLK],
                       start=(m == 0), stop=(m == 2))
                tg = sc.tile([2, BLK], BF16, tag="tg")
                nc.scalar.activation(tg[:], g_ps[0:2, :], AF.Tanh,
                                     bias=W["gbdh"][0:2, 0:1], scale=0.5)
                st["tg"] = tg

            def phase4b(st, b):
                r0 = (b % nblk) * BLK
                tg, x2all = st["tg"], st["x2all"]
                num = sc.tile([2, BLK], BF16, tag="gnum")
                nc.vector.tensor_scalar_add(num[:], tg[:], 1.0)
                den = sc.tile([2, BLK], BF16, tag="gden")
                nc.vector.tensor_scalar(den[:], tg[:], -1.0, 1.0,
                                        AL.mult, AL.add)
                rden = sc.tile([2, BLK], BF16, tag="grden")
                nc.vector.reciprocal(rden[:], den[:])
                nc.vector.tensor_tensor(e2ext[0:2, :], num[:], rden[:],
                                        AL.mult)
                Z_ps = psp.tile([D, BLK], F32, tag="ps")
                mm(Z_ps[0:32, :], W["ones3z"][0:NB, :], e2ext[0:NB, :])
                rz = sc.tile([2, BLK], BF16, tag="rz")
                nc.vector.reciprocal(rz[:], Z_ps[0:2, :])
                a3 = sc.tile([2, BLK], BF16, tag="a3")
                nc.vector.tensor_tensor(a3[:], e2ext[0:2, :], rz[:], AL.mult)
                ab_ps = []
                for n in range(2):
                    abp = psp.tile([D, BLK], F32, tag="ps")
                    mm(abp[:], W["esel"][0:2, n * D:(n + 1) * D], a3[:])
                    ab_ps.append(abp)
                x2_0 = x2all[:, 0 * BLK:1 * BLK]
                x2_1 = x2all[:, 1 * BLK:2 * BLK]
                x2_2 = x2all[:, 2 * BLK:3 * BLK]
                d0 = sc.tile([D, BLK], BF16, tag="gd0")
                nc.vector.tensor_tensor(d0[:], x2_0, x2_2, AL.subtract)
                d1 = sc.tile([D, BLK], BF16, tag="gd1")
                nc.vector.tensor_tensor(d1[:], x2_1, x2_2, AL.subtract)
                m0 = sc.tile([D, BLK], BF16, tag="gm0")
                nc.vector.tensor_tensor(m0[:], d0[:], ab_ps[0][:], AL.mult)
                f1 = sc.tile([D, BLK], BF16, tag="gf1")
                nc.vector.tensor_tensor(f1[:], m0[:], x2_2, AL.add)
                m1 = sc.tile([D, BLK], BF16, tag="gm1")
                nc.vector.tensor_tensor(m1[:], d1[:], ab_ps[1][:], AL.mult)
                fused = sc.tile([D, BLK], BF16, tag="gfused")
                nc.vector.tensor_tensor(fused[:], f1[:], m1[:], AL.add)
                nc.gpsimd.dma_start(out[:, r0:r0 + BLK], fused[:])

            total = nblk * repeat
            bstate = {0: phase0(0)}
            for t in range(total + 3):
                # A-group: gelu/tanh/identity table set
                if 0 <= t - 2 < total:
                    phase3a(bstate[t - 2])
                if 0 <= t - 1 < total:
                    phase2a(bstate[t - 1])
                if 0 <= t - 3 < total:
                    phase4a(bstate[t - 3])
                # B-group: reciprocal_sqrt/square table set
                if 0 <= t - 1 < total:
                    phase2b(bstate[t - 1])
                if t < total:
                    phase1(bstate[t])
                if 0 <= t - 2 < total:
                    phase3b(bstate[t - 2])
                if 0 <= t - 3 < total:
                    phase4b(bstate.pop(t - 3), t - 3)
                if t + 1 < total:
                    bstate[t + 1] = phase0(t + 1)
    _fix_wait_overflow(nc)
    return nc


def kernel(**inputs):
    _patch_tile_drain()
    B = inputs["x_spatial"].shape[0]
    Bc = B // NCORES
    w = prep_weights(inputs)
    nc = build_program(Bc)
    xb = {k: np.ascontiguousarray(inputs[k]).astype(NPBF)
          for k in ("x_spatial", "x_gradient", "x_frequency")}
    in_maps = []
    for c in range(NCORES):
        m = dict(w)
        for k in ("x_spatial", "x_gradient", "x_frequency"):
            m[k] = np.ascontiguousarray(xb[k][c * Bc:(c + 1) * Bc])
        in_maps.append(m)
    res = run_bass_kernel_spmd(nc, in_maps, list(range(NCORES)))
    outs = [res.results[c]["outT"] for c in range(NCORES)]
    full = np.concatenate([o.T for o in outs], axis=0)
    return np.ascontiguousarray(full.astype(np.float32))


# revision 3
# speedup vs baseline: 2.2962x; 2.2962x over previous
"""Trainium2 Bass kernel for nn_CMAF (cross-modal attention fusion block), v2.

Feature-major layout: [128 features x 1024 samples] tiles, weight-stationary
bf16 matmuls. Key deltas vs v1:
 - LayerNorm rsqrt via single-op AF.Rsqrt (reciprocal_sqrt table set); the
   two ACT table sets per tick are {gelu,tanh,identity,square} and
   {reciprocal_sqrt,square,identity} -> still 2 table loads per tick.
 - attention 2-way softmax: a0 = 0.5 + 0.5*tanh(d*ISQ/2) (tanh lives in the
   gelu set), d accumulated by 12 packed 32x32 tile_position matmuls
   (per-head reduce+broadcast), with v/out-proj weights partition-rolled by
   32n per branch so each branch's head sums land in distinct array columns.
 - out-projection runs once on (a0*dv + v1).
 - FFN first matmul consumes x1 directly (no rb2 rescale of W1 outputs).
 - gate softmax over differences d_n = g_n - g_2: e^d via tanh identity
   e^d = (1+t)/(1-t), fused = x2_2 + a0*(x2_0-x2_2) + a1*(x2_1-x2_2).
 - output written feature-major bf16 [128, Bc] to DRAM; transposed and
   upcast on host.

Data parallel over 8 NeuronCores: 8192 samples each.
"""

import numpy as np
import ml_dtypes

import concourse.bass as bass
import concourse.mybir as mybir
from concourse.tile import TileContext
from concourse.vector_clock import ScopedClock
from concourse.bass_utils import run_bass_kernel_spmd

F32 = mybir.dt.float32
BF16 = mybir.dt.bfloat16
AL = mybir.AluOpType
AF = mybir.ActivationFunctionType
NPBF = ml_dtypes.bfloat16

D = 128
SP = 1280
FFN = 256
NB = 3
DH = 32
KV_IDX = ((1, 2), (0, 2), (0, 1))
NCORES = 8
BLK = 1024
MMN = 512
EPS = 1e-5
ISQ = float(1.0 / np.sqrt(DH))


def _patch_tile_drain():
    """walrus rejects >4 sem waits on one instruction; Tile's tail drain
    carries one wait per logical proc. Re-emit as standalone wait_ge."""
    TC = TileContext
    if getattr(TC, "_drain_patched", False):
        return

    def patched(self, tick_clock, wait_clock):
        nop_inst = self.nc.sync.nop()
        wait_clock.add_sem_waits(
            nop_inst.ins, ScopedClock({None: tick_clock.global_clock})
        )
        d = nop_inst.ins
        si = d.sync_info
        waits = list(si.on_wait) if si is not None else []
        if len(waits) > 4:
            si.on_wait = []
            d.sync_info = si
            name2sem = {s.name: s for s in self.sems.allocated().values()}
            for w in waits:
                sem = name2sem.get(w.ant_name)
                if sem is None:
                    raise RuntimeError(f"drain patch: unknown sem {w.ant_name}")
                self.nc.sync.wait_ge(sem, w.wait_value)
        self.nc.sync.drain()
        self.nc.all_engine_barrier()
        popped = self.nc._tile_sem_poison_stack.pop()
        assert popped is self._sem_poison
        self.nc.clear_and_free_semaphores(list(self.sems.allocated().values()))
        self.nc.all_engine_barrier()

    TC._drain_and_barrier = patched
    TC._drain_patched = True


def _fix_wait_overflow(nc):
    """Move excess sync-waits onto same-engine NOPs before the instruction."""
    LIMITS = {}
    DEFAULT_LIM = 1
    for fn in nc.m.functions:
        for bb in fn.blocks:
            insts = list(bb.instructions)
            out = []
            changed = False
            for inst in insts:
                si = getattr(inst, "sync_info", None)
                w = list(si.on_wait) if si is not None and si.on_wait else []
                lim = LIMITS.get(type(inst).__name__, DEFAULT_LIM)
                if len(w) > lim:
                    excess = w[lim:]
                    keep = w[:lim]
                    eng = nc.engines[inst.engine]
                    nops = []
                    for i in range(0, len(excess), 1):
                        chunk = excess[i:i + 1]
                        nop_bi = eng.nop()
                        nop_inst = nop_bi.ins
                        cb = nc.cur_bb.bb
                        cb.instructions = [x for x in cb.instructions
                                           if x.name != nop_inst.name]
                        import bass_rust
                        nop_inst.sync_info = bass_rust.SyncInfo(
                            on_wait=chunk, on_update=[])
                        nops.append(nop_inst)
                    si.on_wait = keep
                    inst.sync_info = si
                    out.extend(nops)
                    changed = True
                out.append(inst)
            if changed:
                bb.instructions = out


def prep_weights(inp):
    f64 = np.float64
    C = np.eye(D, dtype=f64) - 1.0 / D

    def bf(a):
        return np.ascontiguousarray(np.asarray(a, dtype=np.float32)).astype(NPBF)

    def f32(a):
        return np.ascontiguousarray(np.asarray(a), dtype=np.float32)

    w = {}
    wsp = C @ np.asarray(inp["proj_w_spatial"], f64)          # [128,1280]
    w["wspT"] = bf(np.concatenate(
        [wsp[:, c * D:(c + 1) * D].T for c in range(10)], axis=1))
    wgf = np.stack([C @ np.asarray(inp["proj_w_gf"], f64)[i] for i in range(2)])
    w["wgfT"] = bf(np.concatenate([wgf[i].T for i in range(2)], axis=1))
    w["bc"] = f32((C @ np.asarray(inp["proj_b"], f64).T))      # [128,3]

    emb = np.asarray(inp["mod_emb"], f64)                      # [3,128]
    emb_c = emb - emb.mean(axis=1, keepdims=True)

    ipw = np.asarray(inp["in_proj_w"], f64)                    # [3, 384, 128]
    Wq = ipw[:, :D]
    Wk = ipw[:, D:2 * D]
    Wv = ipw[:, 2 * D:]
    ow = np.asarray(inp["out_proj_w"], f64)                    # [3,128,128]
    ob = np.asarray(inp["out_proj_b"], f64)                    # [3,128]

    w["wqT"] = bf(np.concatenate([Wq[n].T for n in range(NB)], axis=1))
    w["wkT"] = bf(np.concatenate([Wk[n].T for n in range(NB)], axis=1))
    w["wvT"] = bf(np.concatenate(
        [np.roll(Wv[n], 32 * n, axis=0).T for n in range(NB)], axis=1))
    owc = [C @ ow[n] for n in range(NB)]
    w["owT"] = bf(np.concatenate(
        [np.roll(owc[n], 32 * n, axis=1).T for n in range(NB)], axis=1))

    w["qb"] = f32(np.stack([Wq[n] @ emb[n] for n in range(NB)], axis=1))
    w["demb"] = f32(np.stack(
        [emb[KV_IDX[n][0]] - emb[KV_IDX[n][1]] for n in range(NB)], axis=1))
    ube = [C @ ob[n] + C @ (ow[n] @ (Wv[n] @ emb[KV_IDX[n][1]])) + emb_c[n]
           for n in range(NB)]
    w["ube"] = f32(np.stack(ube, axis=1))

    w1 = np.asarray(inp["ffn_w1"], f64)                        # [3,256,128]
    w["w1T"] = bf(np.concatenate(
        [w1[n][c * D:(c + 1) * D].T for n in range(NB) for c in range(2)],
        axis=1))
    w["b1"] = f32(np.asarray(inp["ffn_b1"], f64).reshape(NB * 2, D).T)
    w2 = [C @ np.asarray(inp["ffn_w2"], f64)[n] for n in range(NB)]  # [128,256]
    w["w2T"] = bf(np.concatenate(
        [w2[n][:, c * D:(c + 1) * D].T for n in range(NB) for c in range(2)],
        axis=1))
    w["b2c"] = f32(np.stack([C @ np.asarray(inp["ffn_b2"], f64)[n]
                             for n in range(NB)], axis=1))

    gw = np.asarray(inp["gate_w"], f64).reshape(NB, NB, D)     # [out n, m, d]
    gb = np.asarray(inp["gate_b"], f64)
    gwd = np.stack([gw[n] - gw[2] for n in range(2)])          # [2, 3, 128]
    w["gwdT"] = bf(np.concatenate(
        [gwd[:, m, :].T for m in range(NB)], axis=1))          # [128, 6]
    w["gbdh"] = f32(0.5 * (gb[:2] - gb[2]).reshape(2, 1))      # tanh bias

    w["onesT"] = bf(np.full((D, D), 1.0 / D))
    w["hones"] = bf(np.ones((D, 32)))
    w["ones3z"] = bf(np.ones((NB, 32)))
    esel = np.zeros((2, 2 * D), dtype=np.float32)
    esel[0, :D] = 1.0
    esel[1, D:] = 1.0
    w["esel"] = bf(esel)
    w["epsv"] = np.full((D, 1), EPS, dtype=np.float32)
    w["zerov"] = np.zeros((D, 1), dtype=np.float32)

    assert np.allclose(inp["proj_ln_g"], 1) and np.allclose(inp["proj_ln_b"], 0)
    assert np.allclose(inp["attn_ln_g"], 1) and np.allclose(inp["attn_ln_b"], 0)
    assert np.allclose(inp["ffn_ln_g"], 1) and np.allclose(inp["ffn_ln_b"], 0)
    assert np.allclose(inp["in_proj_b"], 0)
    return w


WEIGHT_SPECS = {
    "wspT": ((D, 10 * D), BF16), "wgfT": ((D, 2 * D), BF16),
    "bc": ((D, NB), F32),
    "wqT": ((D, NB * D), BF16), "wkT": ((D, NB * D), BF16),
    "wvT": ((D, NB * D), BF16), "owT": ((D, NB * D), BF16),
    "qb": ((D, NB), F32), "demb": ((D, NB), F32), "ube": ((D, NB), F32),
    "w1T": ((D, NB * 2 * D), BF16), "b1": ((D, NB * 2), F32),
    "w2T": ((D, NB * 2 * D), BF16), "b2c": ((D, NB), F32),
    "gwdT": ((D, 2 * NB), BF16), "gbdh": ((2, 1), F32),
    "onesT": ((D, D), BF16), "hones": ((D, 32), BF16),
    "ones3z": ((NB, 32), BF16), "esel": ((2, 2 * D), BF16),
    "epsv": ((D, 1), F32), "zerov": ((D, 1), F32),
}


def build_program(Bc, repeat=1):
    nc = bass.Bass()
    xsp = nc.dram_tensor("x_spatial", [Bc, SP], BF16, kind="ExternalInput")
    xg = nc.dram_tensor("x_gradient", [Bc, D], BF16, kind="ExternalInput")
    xf = nc.dram_tensor("x_frequency", [Bc, D], BF16, kind="ExternalInput")
    wd = {k: nc.dram_tensor(k, list(s[0]), s[1], kind="ExternalInput")
          for k, s in WEIGHT_SPECS.items()}
    # feature-major output; host transposes
    out = nc.dram_tensor("outT", [D, Bc], BF16, kind="ExternalOutput")

    nblk = Bc // BLK
    assert Bc % BLK == 0

    with TileContext(nc) as tc, nc.allow_low_precision(reason="bf16 kernel"):
        with (
            tc.tile_pool(name="wp", bufs=1) as wp,
            tc.tile_pool(name="xin", bufs=2) as xin,
            tc.tile_pool(name="pp", bufs=2) as pp,
            tc.tile_pool(name="sc", bufs=1) as sc,
            tc.tile_pool(name="sc2", bufs=2) as sc2,
            tc.tile_pool(name="ps", bufs=4, space="PSUM") as psp,
        ):
            W = {}
            for k, s in WEIGHT_SPECS.items():
                W[k] = wp.tile(list(s[0]), s[1], tag=k, name=k)
                nc.gpsimd.dma_start(W[k][:], wd[k][:])
            # persistent gate-exp tile: row 2 stays 1.0
            e2ext = wp.tile([NB, BLK], BF16, tag="e2ext", name="e2ext")
            nc.vector.memset(e2ext[:], 1.0)

            def act_rsqrt(out_ap, in_ap, bias):
                bi = nc.scalar.activation(out_ap, in_ap, AF.Sqrt, bias=bias)
                bi.ins.func = AF.Rsqrt
                return bi

            def mm(out_ap, lhsT, rhs, start=True, stop=True):
                for h in range(BLK // MMN):
                    nc.tensor.matmul(out_ap[:, h * MMN:(h + 1) * MMN], lhsT,
                                     rhs[:, h * MMN:(h + 1) * MMN],
                                     start=start, stop=stop)

            def phase0(b):
                r0 = (b % nblk) * BLK
                st = {}
                xspT = xin.tile([D, 10 * BLK], BF16, tag="xspT")
                nc.sync.dma_start(
                    xspT[:].rearrange("p (c n) -> p c n", c=10),
                    xsp[r0:r0 + BLK, :], transpose=True)
                st["xspT"] = xspT
                st["xgT"] = xin.tile([D, BLK], BF16, tag="xgT")
                nc.sync.dma_start(st["xgT"][:], xg[r0:r0 + BLK, :], transpose=True)
                st["xfT"] = xin.tile([D, BLK], BF16, tag="xfT")
                nc.sync.dma_start(st["xfT"][:], xf[r0:r0 + BLK, :], transpose=True)
                return st

            def phase1(st):
                # projections + LN1 -> Pall (no emb), dPall (emb diff folded)
                z_ps = []
                zs = psp.tile([D, BLK], F32, tag="ps")
                for c in range(10):
                    mm(zs[:], W["wspT"][:, c * D:(c + 1) * D],
                       st["xspT"][:, c * BLK:(c + 1) * BLK],
                       start=(c == 0), stop=(c == 9))
                z_ps.append(zs)
                for i, key in ((0, "xgT"), (1, "xfT")):
                    zt = psp.tile([D, BLK], F32, tag="ps")
                    mm(zt[:], W["wgfT"][:, i * D:(i + 1) * D], st[key][:])
                    z_ps.append(zt)
                Pall = pp.tile([D, NB * BLK], BF16, tag="Pall")
                sqa = sc.tile([D, BLK], BF16, tag="sq1")
                for n in range(NB):
                    nc.scalar.activation(sqa[:], z_ps[n][:], AF.Square,
                                         bias=W["bc"][:, n:n + 1])
                    mq = psp.tile([D, BLK], F32, tag="ps")
                    mm(mq[:], W["onesT"][:], sqa[:])
                    rb = sc.tile([D, BLK], BF16, tag=f"rb1_{n}")
                    nc.scalar.activation(rb[:], mq[:], AF.Rsqrt,
                                         bias=W["epsv"][:, 0:1])
                    nc.vector.scalar_tensor_tensor(
                        Pall[:, n * BLK:(n + 1) * BLK], z_ps[n][:],
                        W["bc"][:, n:n + 1], rb[:], AL.add, AL.mult)
                dPall = pp.tile([D, NB * BLK], BF16, tag="dPall")
                for n in range(NB):
                    s0, s1 = KV_IDX[n]
                    nc.vector.scalar_tensor_tensor(
                        dPall[:, n * BLK:(n + 1) * BLK],
                        Pall[:, s0 * BLK:(s0 + 1) * BLK],
                        W["demb"][:, n:n + 1],
                        Pall[:, s1 * BLK:(s1 + 1) * BLK],
                        AL.add, AL.subtract)
                st["Pall"] = Pall
                st["dPall"] = dPall

            def phase2a(st):
                Pall, dPall = st["Pall"], st["dPall"]
                uall = sc.tile([D, NB * BLK], BF16, tag="uall")
                for n in range(NB):
                    Pn = Pall[:, n * BLK:(n + 1) * BLK]
                    dPn = dPall[:, n * BLK:(n + 1) * BLK]
                    s1 = KV_IDX[n][1]
                    q_ps = psp.tile([D, BLK], F32, tag="ps")
                    mm(q_ps[:], W["wqT"][:, n * D:(n + 1) * D], Pn)
                    dk_ps = psp.tile([D, BLK], F32, tag="ps")
                    mm(dk_ps[:], W["wkT"][:, n * D:(n + 1) * D], dPn)
                    qx = sc.tile([D, BLK], BF16, tag="qx")
                    nc.scalar.activation(qx[:], q_ps[:], AF.Identity,
                                         bias=W["qb"][:, n:n + 1])
                    t0 = sc.tile([D, BLK], BF16, tag="t0")
                    nc.vector.tensor_tensor(t0[:], qx[:], dk_ps[:], AL.mult)
                    d_ps = psp.tile([D, BLK], F32, tag="ps")
                    for h in range(4):
                        c = (h + n) % 4
                        for k in range(BLK // MMN):
                            nc.tensor.matmul(
                                d_ps[32 * c:32 * c + 32, k * MMN:(k + 1) * MMN],
                                W["hones"][32 * h:32 * h + 32, :],
                                t0[32 * h:32 * h + 32, k * MMN:(k + 1) * MMN],
                                start=True, stop=True,
                                tile_position=(32 * h, 32 * c))
                    th = sc.tile([D, BLK], BF16, tag="th")
                    nc.scalar.activation(th[:], d_ps[:], AF.Tanh,
                                         bias=W["zerov"][:, 0:1],
                                         scale=ISQ * 0.5)
                    a0 = sc.tile([D, BLK], BF16, tag="a0")
                    nc.vector.tensor_scalar(a0[:], th[:], 0.5, 0.5,
                                            AL.mult, AL.add)
                    dv_ps = psp.tile([D, BLK], F32, tag="ps")
                    mm(dv_ps[:], W["wvT"][:, n * D:(n + 1) * D], dPn)
                    v1_ps = psp.tile([D, BLK], F32, tag="ps")
                    mm(v1_ps[:], W["wvT"][:, n * D:(n + 1) * D],
                       Pall[:, s1 * BLK:(s1 + 1) * BLK])
                    tp = sc.tile([D, BLK], BF16, tag="tp")
                    nc.vector.tensor_tensor(tp[:], a0[:], dv_ps[:], AL.mult)
                    tpv = sc.tile([D, BLK], BF16, tag="tpv")
                    nc.vector.tensor_tensor(tpv[:], tp[:], v1_ps[:], AL.add)
                    o_ps = psp.tile([D, BLK], F32, tag="ps")
                    mm(o_ps[:], W["owT"][:, n * D:(n + 1) * D], tpv[:])
                    nc.vector.scalar_tensor_tensor(
                        uall[:, n * BLK:(n + 1) * BLK], o_ps[:],
                        W["ube"][:, n:n + 1], Pn, AL.add, AL.add)
                st["uall"] = uall

            def phase2b(st):
                uall = st["uall"]
                squ = sc.tile([D, NB * BLK], BF16, tag="squ")
                nc.scalar.activation(squ[:], uall[:], AF.Square,
                                     bias=W["zerov"][:, 0:1])
                x1all = sc2.tile([D, NB * BLK], BF16, tag="x1all")
                rb2 = sc.tile([D, NB * BLK], BF16, tag="rb2")
                for n in range(NB):
                    mq = psp.tile([D, BLK], F32, tag="ps")
                    mm(mq[:], W["onesT"][:], squ[:, n * BLK:(n + 1) * BLK])
                    act_rsqrt(rb2[:, n * BLK:(n + 1) * BLK], mq[:],
                              W["epsv"][:, 0:1])
                    nc.vector.tensor_tensor(
                        x1all[:, n * BLK:(n + 1) * BLK],
                        uall[:, n * BLK:(n + 1) * BLK],
                        rb2[:, n * BLK:(n + 1) * BLK], AL.mult)
                st["x1all"] = x1all

            def phase3a(st):
                x1all = st["x1all"]
                gsb = sc.tile([D, NB * 2 * BLK], BF16, tag="gsb")
                for n in range(NB):
                    x1n = x1all[:, n * BLK:(n + 1) * BLK]
                    for c in range(2):
                        j = 2 * n + c
                        h_ps = psp.tile([D, BLK], F32, tag="ps")
                        mm(h_ps[:], W["w1T"][:, j * D:(j + 1) * D], x1n)
                        nc.scalar.activation(gsb[:, j * BLK:(j + 1) * BLK],
                                             h_ps[:], AF.Gelu,
                                             bias=W["b1"][:, j:j + 1])
                st["gsb"] = gsb

            def phase3b(st):
                x1all, gsb = st["x1all"], st["gsb"]
                x2all = sc2.tile([D, NB * BLK], BF16, tag="x2all")
                sq3 = sc.tile([D, BLK], BF16, tag="sq3")
                for n in range(NB):
                    f_ps = psp.tile([D, BLK], F32, tag="ps")
                    for c in range(2):
                        j = 2 * n + c
                        mm(f_ps[:], W["w2T"][:, j * D:(j + 1) * D],
                           gsb[:, j * BLK:(j + 1) * BLK],
                           start=(c == 0), stop=(c == 1))
                    x2p = sc.tile([D, BLK], BF16, tag=f"x2p{n}")
                    nc.vector.tensor_tensor(
                        x2p[:], x1all[:, n * BLK:(n + 1) * BLK], f_ps[:],
                        AL.add)
                    nc.scalar.activation(sq3[:], x2p[:], AF.Square,
                                         bias=W["b2c"][:, n:n + 1])
                    mq = psp.tile([D, BLK], F32, tag="ps")
                    mm(mq[:], W["onesT"][:], sq3[:])
                    rb3 = sc.tile([D, BLK], BF16, tag="rb3")
                    nc.scalar.activation(rb3[:], mq[:], AF.Rsqrt,
                                         bias=W["epsv"][:, 0:1])
                    nc.vector.scalar_tensor_tensor(
                        x2all[:, n * BLK:(n + 1) * BLK], x2p[:],
                        W["b2c"][:, n:n + 1], rb3[:], AL.add, AL.mult)
                st["x2all"] = x2all

            def phase4a(st):
                x2all = st["x2all"]
                g_ps = psp.tile([D, BLK], F32, tag="ps")
                for m in range(NB):
                    mm(g_ps[0:2, :], W["gwdT"][:, m * 2:(m + 1) * 2],
                       x2all[:, m * BLK:(m + 1) * BLK],
                       start=(m == 0), stop=(m == 2))
                tg = sc.tile([2, BLK], BF16, tag="tg")
                nc.scalar.activation(tg[:], g_ps[0:2, :], AF.Tanh,
                                     bias=W["gbdh"][0:2, 0:1], scale=0.5)
                st["tg"] = tg

            def phase4b(st, b):
                r0 = (b % nblk) * BLK
                tg, x2all = st["tg"], st["x2all"]
                num = sc.tile([2, BLK], BF16, tag="gnum")
                nc.vector.tensor_scalar_add(num[:], tg[:], 1.0)
                den = sc.tile([2, BLK], BF16, tag="gden")
                nc.vector.tensor_scalar(den[:], tg[:], -1.0, 1.0,
                                        AL.mult, AL.add)
                rden = sc.tile([2, BLK], BF16, tag="grden")
                nc.vector.reciprocal(rden[:], den[:])
                nc.vector.tensor_tensor(e2ext[0:2, :], num[:], rden[:],
                                        AL.mult)
                Z_ps = psp.tile([D, BLK], F32, tag="ps")
                mm(Z_ps[0:32, :], W["ones3z"][0:NB, :], e2ext[0:NB, :])
                rz = sc.tile([2, BLK], BF16, tag="rz")
                nc.vector.reciprocal(rz[:], Z_ps[0:2, :])
                a3 = sc.tile([2, BLK], BF16, tag="a3")
                nc.vector.tensor_tensor(a3[:], e2ext[0:2, :], rz[:], AL.mult)
                ab_ps = []
                for n in range(2):
                    abp = psp.tile([D, BLK], F32, tag="ps")
                    mm(abp[:], W["esel"][0:2, n * D:(n + 1) * D], a3[:])
                    ab_ps.append(abp)
                x2_0 = x2all[:, 0 * BLK:1 * BLK]
                x2_1 = x2all[:, 1 * BLK:2 * BLK]
                x2_2 = x2all[:, 2 * BLK:3 * BLK]
                d0 = sc.tile([D, BLK], BF16, tag="gd0")
                nc.vector.tensor_tensor(d0[:], x2_0, x2_2, AL.subtract)
                d1 = sc.tile([D, BLK], BF16, tag="gd1")
                nc.vector.tensor_tensor(d1[:], x2_1, x2_2, AL.subtract)
                m0 = sc.tile([D, BLK], BF16, tag="gm0")
                nc.vector.tensor_tensor(m0[:], d0[:], ab_ps[0][:], AL.mult)
                f1 = sc.tile([D, BLK], BF16, tag="gf1")
                nc.vector.tensor_tensor(f1[:], m0[:], x2_2, AL.add)
                m1 = sc.tile([D, BLK], BF16, tag="gm1")
                nc.vector.tensor_tensor(m1[:], d1[:], ab_ps[1][:], AL.mult)
                fused = sc.tile([D, BLK], BF16, tag="gfused")
                nc.vector.tensor_tensor(fused[:], f1[:], m1[:], AL.add)
                nc.gpsimd.dma_start(out[:, r0:r0 + BLK], fused[:])

            total = nblk * repeat
            bstate = {0: phase0(0)}
            for t in range(total + 3):
                # A-group: gelu/tanh/identity table set
                if 0 <= t - 2 < total:
                    phase3a(bstate[t - 2])
                if 0 <= t - 1 < total:
                    phase2a(bstate[t - 1])
                if 0 <= t - 3 < total:
                    phase4a(bstate[t - 3])
                # B-group: reciprocal_sqrt/square table set
                if 0 <= t - 1 < total:
                    phase2b(bstate[t - 1])
                if t < total:
                    phase1(bstate[t])
                if 0 <= t - 2 < total:
                    phase3b(bstate[t - 2])
                if 0 <= t - 3 < total:
                    phase4b(bstate.pop(t - 3), t - 3)
                if t + 1 < total:
                    bstate[t + 1] = phase0(t + 1)
    _fix_wait_overflow(nc)
    return nc


def kernel(**inputs):
    _patch_tile_drain()
    B = inputs["x_spatial"].shape[0]
    Bc = B // NCORES
    w = prep_weights(inputs)
    nc = build_program(Bc)
    xb = {k: np.ascontiguousarray(inputs[k]).astype(NPBF)
          for k in ("x_spatial", "x_gradient", "x_frequency")}
    in_maps = []
    for c in range(NCORES):
        m = dict(w)
        for k in ("x_spatial", "x_gradient", "x_frequency"):
            m[k] = np.ascontiguousarray(xb[k][c * Bc:(c + 1) * Bc])
        in_maps.append(m)
    res = run_bass_kernel_spmd(nc, in_maps, list(range(NCORES)))
    outs = [res.results[c]["outT"] for c in range(NCORES)]
    full = np.concatenate([o.T for o in outs], axis=0)
    return np.ascontiguousarray(full.astype(np.float32))
